# revision 1
# baseline (speedup 1.0000x reference)
import os
import sys

sys.path.insert(0, "/opt/trn_rl_repo")

import numpy as np

# ---------------------------------------------------------------- problem dims
NCORES = 8
N = 50000
E = 800000
IN_F, HID_F, OUT_F = 256, 128, 64
NEG = 0.2
EPS = 1e-16

NPC = N // NCORES            # 6250 nodes (= targets) per core
BPB = 128                    # targets per block
NB = (NPC + BPB - 1) // BPB  # 49 blocks per core
ROWS = NB * BPB              # 6272 padded rows per core slice
TBL = NCORES * ROWS          # 50176 rows in the all-gathered table
TH = 32768                   # int16 gather index threshold
CHUNK_SLOTS = 8              # max 128-edge slots per chunk (dma_gather <=1024 idxs/call)
WSHIFT = 8.0                 # global exp shift (cancels in normalization)


def _householder(a):
    """Symmetric orthogonal H with (H h)[0] == (a/||a||) . h ; returns H, ||a||."""
    a = np.asarray(a, dtype=np.float64)
    d = a.shape[0]
    alpha = np.linalg.norm(a)
    u = a.copy()
    # map a -> sign * alpha * e0 (numerically stable choice)
    sgn = 1.0 if a[0] >= 0 else -1.0
    u[0] += sgn * alpha
    nu = np.linalg.norm(u)
    Hm = np.eye(d) - 2.0 * np.outer(u, u) / (nu * nu)
    # H @ a = -sgn*alpha*e0  =>  (H h)[0] = -sgn * (a.h)/alpha; fold sign into c
    return Hm.astype(np.float32), np.float32(-sgn * alpha)


def prep_structures(edge_index):
    """Host-side layout of the edge list. Uniform (across cores) compile-time
    structure: per block, lo/hi slot counts; per chunk, gather-call geometry.
    Returns meta dict + per-core numpy arrays."""
    src = edge_index[0].astype(np.int64)
    tgt = edge_index[1].astype(np.int64)
    adj = (src // NPC) * ROWS + (src % NPC)  # row in all-gathered table

    order = np.argsort(tgt, kind="stable")
    src_a = adj[order]
    tgt_s = tgt[order]

    core_of = tgt_s // NPC
    blk_of = (tgt_s % NPC) // BPB
    rel_of = (tgt_s % NPC) % BPB
    gb = core_of * NB + blk_of
    # edges are sorted by tgt so gb is non-decreasing
    bounds = np.searchsorted(gb, np.arange(NCORES * NB + 1))

    # per (core, block): lo/hi edge lists
    lo_cnt = np.zeros((NCORES, NB), dtype=np.int64)
    hi_cnt = np.zeros((NCORES, NB), dtype=np.int64)
    per_kb = {}
    for k in range(NCORES):
        for b in range(NB):
            g = k * NB + b
            s, e = bounds[g], bounds[g + 1]
            sa = src_a[s:e]
            rl = rel_of[s:e]
            m = sa < TH
            lo_o = np.argsort(sa[m], kind="stable")
            hi_o = np.argsort(sa[~m], kind="stable")
            per_kb[(k, b)] = (sa[m][lo_o], rl[m][lo_o], sa[~m][hi_o], rl[~m][hi_o])
            lo_cnt[k, b] = int(m.sum())
            hi_cnt[k, b] = int((~m).sum())

    nlo = np.maximum(1, np.ceil(lo_cnt.max(axis=0) / 128.0)).astype(np.int64)
    nhi = np.ceil(hi_cnt.max(axis=0) / 128.0).astype(np.int64)  # may be 0
    ns = nlo + nhi  # slots per block (uniform)
    s_off = np.concatenate([[0], np.cumsum(ns)])  # slot offset per block
    S_TOT = int(s_off[-1])

    # per-core grids: SRC (gather row, 0-padded), REL (target-rel, -1 padded)
    SRC = np.zeros((NCORES, 128, S_TOT), dtype=np.int64)
    REL = np.full((NCORES, 128, S_TOT), -1.0, dtype=np.float32)
    for k in range(NCORES):
        for b in range(NB):
            o = int(s_off[b])
            la, lr, ha, hr = per_kb[(k, b)]
            for (arr, rel, base, cnt) in (
                (la, lr, o, int(nlo[b])),
                (ha, hr, o + int(nlo[b]), int(nhi[b])),
            ):
                n = len(arr)
                if n == 0:
                    continue
                full = np.zeros(cnt * 128, dtype=np.int64)
                full[:n] = arr
                fr = np.full(cnt * 128, -1.0, dtype=np.float32)
                fr[:n] = rel
                SRC[k, :, base:base + cnt] = full.reshape(cnt, 128).T
                REL[k, :, base:base + cnt] = fr.reshape(cnt, 128).T
    # hi slots hold (row - TH) for the offset table view; pads must stay valid
    # for whichever call they land in (lo call -> 0 ok; hi call -> 0 maps to
    # row TH which exists). handled below when emitting per-call indices.

    # chunk/call structure per block (uniform across cores)
    # chunk: (slot_lo, n_slots, calls) ; call: (slot_in_chunk, n_slots, is_hi)
    blocks = []
    for b in range(NB):
        chunks = []
        c0 = 0
        while c0 < int(ns[b]):
            cs = min(CHUNK_SLOTS, int(ns[b]) - c0)
            calls = []
            lo_end = int(nlo[b])
            a0, a1 = c0, min(c0 + cs, lo_end)
            if a1 > a0:
                calls.append((a0 - c0, a1 - a0, False))
            h0, h1 = max(c0, lo_end), c0 + cs
            if h1 > h0:
                calls.append((h0 - c0, h1 - h0, True))
            chunks.append((c0, cs, calls))
            c0 += cs
        blocks.append(chunks)

    # emit per-call wrapped int16 index arrays, concatenated along columns
    idx_parts = [[] for _ in range(NCORES)]
    call_cols = []  # (col_off, n_cols) per call, in emission order
    col_off = 0
    for b in range(NB):
        for (c0, cs, calls) in blocks[b]:
            for (sic, ncall, is_hi) in calls:
                s0 = int(s_off[b]) + c0 + sic
                n_idx = ncall * 128
                cols = n_idx // 16
                call_cols.append((col_off, cols, n_idx))
                col_off += cols
                for k in range(NCORES):
                    vals = SRC[k][:, s0:s0 + ncall].flatten(order="F")
                    if is_hi:
                        vals = np.maximum(vals - TH, 0)
                    w16 = vals.reshape(-1, 16).T  # [16, cols]
                    idx_parts[k].append(np.tile(w16, (8, 1)).astype(np.int16))
    eidx = [np.concatenate(idx_parts[k], axis=1) for k in range(NCORES)]

    meta = dict(
        ns=ns, s_off=s_off, S_TOT=S_TOT, blocks=blocks, call_cols=call_cols,
        TOT_COLS=col_off,
    )
    return meta, eidx, REL


# ------------------------------------------------------------------ host model
def host_model(inputs, f16=True):
    """Numpy mirror of the device dataflow (for algorithm validation)."""
    x = np.asarray(inputs["x"], np.float32)
    ei = np.asarray(inputs["edge_index"])
    W1 = np.asarray(inputs["W1"], np.float32)
    b1 = np.asarray(inputs["b1"], np.float32)
    a1w = np.asarray(inputs["a1_w"], np.float32)
    a1b = np.asarray(inputs["a1_b"], np.float32)
    W2 = np.asarray(inputs["W2"], np.float32)
    b2 = np.asarray(inputs["b2"], np.float32)
    a2w = np.asarray(inputs["a2_w"], np.float32)
    a2b = np.asarray(inputs["a2_b"], np.float32)

    meta, eidx, REL = prep_structures(ei)
    R1, c1 = _householder(a1w[:HID_F])
    R2, c2 = _householder(a2w[:OUT_F])
    ed = np.float16 if f16 else np.float32

    def phase1(k):
        xs = np.zeros((ROWS, IN_F), np.float32)
        xs[:NPC] = x[k * NPC:(k + 1) * NPC]
        h = xs @ W1 + b1
        h = np.where(h > 0, h, np.expm1(np.minimum(h, 0.0)))  # elu
        hp = (h @ R1.T).astype(ed)                            # rotated rows
        t1 = h @ a1w[HID_F:] + a1b[0]                         # + bias folded
        return hp, t1.astype(np.float32)

    hp_sl, t1_sl = zip(*[phase1(k) for k in range(NCORES)])
    table1 = np.concatenate(hp_sl, axis=0)  # [TBL, 128]

    def edge_phase(k, table, t_sl, c, d):
        tw = table.shape[1]  # gathered row width (may exceed d via padding)
        out = np.zeros((ROWS, d), np.float32)
        colptr = 0
        iota = np.arange(128, dtype=np.float32)
        for b in range(NB):
            trow = t_sl[k][b * BPB:(b + 1) * BPB]  # [128]
            acc = np.zeros((BPB, d + 1), np.float32)
            for (c0, cs, calls) in meta["blocks"][b]:
                s0 = int(meta["s_off"][b]) + c0
                g = np.zeros((128, cs, tw), ed)
                for (sic, ncall, is_hi) in calls:
                    off, cols, n_idx = meta["call_cols"][colptr]
                    colptr += 1
                    w16 = eidx[k][:16, off:off + cols]
                    flat = w16.T.flatten()[:n_idx].astype(np.int64)
                    if is_hi:
                        flat = flat + TH
                    rows = table[flat].reshape(ncall, 128, tw)
                    g[:, sic:sic + ncall, :] = np.transpose(rows, (1, 0, 2))
                rel = REL[k][:, s0:s0 + cs].astype(ed)  # [128, cs]
                delta = rel[:, :, None] - iota.astype(ed)[None, None, :]
                sel = (delta == 0).astype(ed)
                trow16 = (trow.astype(ed))[None, None, :]
                t_ed = (sel * trow16).sum(axis=2, dtype=np.float32).astype(ed)
                z = (g[:, :, 0].astype(ed) * ed(c) + t_ed).astype(ed)
                zl = np.maximum(z, ed(NEG) * z)
                w = np.exp(zl.astype(np.float32) - WSHIFT).astype(ed)
                Wm = sel * w[:, :, None]
                for j in range(cs):
                    acc[:, :d] += (
                        Wm[:, j, :].astype(np.float32).T
                        @ g[:, j, :d].astype(np.float32)
                    )
                    acc[:, d] += Wm[:, j, :].astype(np.float32).sum(axis=0)
            nrm = acc[:, :d] / (acc[:, d:] + EPS)
            out[b * BPB:(b + 1) * BPB] = nrm
        return out

    # layer 1 edge aggregation, per core, then fused layer-2 prep
    h2p_sl, t2_sl = [], []
    out1_dbg = []
    o1p_dbg = []
    for k in range(NCORES):
        o1p = edge_phase(k, table1, t1_sl, c1, HID_F)  # rotated-basis out
        o1p_dbg.append(o1p)
        o1 = o1p @ R1  # un-rotate (R symmetric: R^T = R); rows [ROWS, 128]
        out1_dbg.append(o1)
        h2 = o1 @ W2 + b2
        h2p = (h2 @ R2.T).astype(ed)
        t2 = h2 @ a2w[OUT_F:] + a2b[0]
        if f16:
            pad = np.zeros((ROWS, 128 - OUT_F), ed)
            h2p = np.concatenate([h2p, pad], axis=1)
        h2p_sl.append(h2p)
        t2_sl.append(t2.astype(np.float32))
    table2 = np.concatenate(h2p_sl, axis=0)
    host_model.table1 = table1
    host_model.table2 = table2
    host_model.t1_sl = t1_sl
    host_model.t2_sl = t2_sl
    host_model.out1 = out1_dbg
    host_model.o1p = o1p_dbg

    outs = []
    for k in range(NCORES):
        o2p = edge_phase(k, table2, t2_sl, c2, OUT_F)
        o2 = o2p @ R2
        m = o2.max(axis=1, keepdims=True)
        lse = np.log(np.exp(o2 - m).sum(axis=1, keepdims=True)) + m
        outs.append((o2 - lse)[:NPC])
    return np.concatenate(outs, axis=0).astype(np.float32)


if __name__ == "__main__":
    sys.path.insert(0, os.path.dirname(os.path.abspath(__file__)))
    import reference

    inputs = {k: np.asarray(v) for k, v in reference.setup_inputs().items()}
    expect = np.asarray(reference.reference(**inputs))
    got = host_model(inputs, f16=True)
    err = np.abs(got - expect)
    rel = err.max() / np.abs(expect).max()
    print("host_model f16: absmax", err.max(), "rel", rel)
    got = host_model(inputs, f16=False)
    err = np.abs(got - expect)
    rel = err.max() / np.abs(expect).max()
    print("host_model f32: absmax", err.max(), "rel", rel)


# ------------------------------------------------------------------ bass build
def _patch_tile_drain():
    """This walrus build supports only one sync-wait per SP TPB_CTRL
    instruction; TileContext's exit drain aggregates the whole global clock
    onto one drain. Split each wait onto its own single-wait NOP first."""
    import concourse.mybir as mybir
    import concourse.tile as tile
    from concourse.tile import ScopedClock

    if getattr(tile.TileContext, "_drain_split_patched", False):
        return

    def _split(self, tick_clock, wait_clock):
        nop0 = self.nc.sync.nop()
        wait_clock.add_sem_waits(
            nop0.ins, ScopedClock({None: tick_clock.global_clock})
        )
        si = nop0.ins.sync_info
        if si is not None and si.on_wait and len(si.on_wait) > 1:
            waits = list(si.on_wait)
            nop0.ins.sync_info = mybir.SyncInfo(
                on_wait=[waits[0]], on_update=list(si.on_update)
            )
            for w in waits[1:]:
                n = self.nc.sync.nop()
                n.ins.sync_info = mybir.SyncInfo(on_wait=[w], on_update=[])
        self.nc.sync.drain()
        self.nc.all_engine_barrier()
        popped = self.nc._tile_sem_poison_stack.pop()
        assert popped is self._sem_poison
        self.nc.clear_and_free_semaphores(list(self.sems.allocated().values()))
        self.nc.all_engine_barrier()

    tile.TileContext._drain_and_barrier = _split
    tile.TileContext._drain_split_patched = True


def _split_multi_waits(nc):
    """This walrus build rejects instructions carrying more than one sync
    wait. Move extra waits onto single-wait NOPs inserted just before the
    instruction (same engine, same per-engine position)."""
    import concourse.mybir as mybir

    ctr = [0]
    for f in nc.m.functions:
        for bb in f.blocks:
            insts = list(bb.instructions)
            out = []
            changed = False
            for ins in insts:
                si = getattr(ins, "sync_info", None)
                if si is not None and si.on_wait and len(si.on_wait) > 1:
                    waits = list(si.on_wait)
                    for w in waits[:-1]:
                        n = mybir.InstNoOp(
                            name=f"splitw-{ctr[0]}", ins=[], outs=[]
                        )
                        ctr[0] += 1
                        n.engine = ins.engine
                        n.sync_info = mybir.SyncInfo(on_wait=[w], on_update=[])
                        nc.register_instruction(n)
                        out.append(n)
                    ins.sync_info = mybir.SyncInfo(
                        on_wait=[waits[-1]], on_update=list(si.on_update)
                    )
                    changed = True
                out.append(ins)
            if changed:
                bb.instructions = out


def build_bass(meta, consts, f16=True):
    import concourse.bass as bass
    import concourse.mybir as mybir
    import concourse.tile as tile
    from concourse.library_config import mlp as mlp_lib
    from concourse.tile_rust import add_dep_helper

    _patch_tile_drain()

    F32 = mybir.dt.float32
    F16 = mybir.dt.float16 if f16 else mybir.dt.float32
    I16 = mybir.dt.int16
    AL = mybir.AluOpType
    AF = mybir.ActivationFunctionType
    AX = mybir.AxisListType
    TW1 = 128                       # table-1 row width (elements)
    TW2 = 128 if f16 else 64        # table-2 row width (fp16 pads to 256B)

    nc = bass.Bass(num_devices=NCORES, num_swdge_queues=4)

    # per-core external inputs
    x_sl = nc.dram_tensor("x_sl", [ROWS, IN_F], F32, kind="ExternalInput")
    eidx = nc.dram_tensor(
        "eidx", [128, meta["TOT_COLS"]], I16, kind="ExternalInput"
    )
    tgtrel = nc.dram_tensor(
        "tgtrel", [128, meta["S_TOT"]], F16, kind="ExternalInput"
    )
    out_fin = nc.dram_tensor("out_fin", [ROWS, OUT_F], F32, kind="ExternalOutput")
    debug = os.environ.get("GNN_DEBUG", "0") == "1"
    if debug:
        dbg_h1 = nc.dram_tensor("dbg_h1", [TBL, 128], F16, kind="ExternalOutput")
        dbg_h2 = nc.dram_tensor("dbg_h2", [TBL, 128 if f16 else 64], F16, kind="ExternalOutput")
        dbg_t1 = nc.dram_tensor("dbg_t1", [1, ROWS], F32, kind="ExternalOutput")
        dbg_nrm = nc.dram_tensor("dbg_nrm", [ROWS, HID_F], F32, kind="ExternalOutput")
        dbg_z = nc.dram_tensor("dbg_z", [128, meta["S_TOT"]], F16, kind="ExternalOutput")
        dbg_ted = nc.dram_tensor("dbg_ted", [128, meta["S_TOT"]], F16, kind="ExternalOutput")
        dbg_w = nc.dram_tensor("dbg_w", [128, meta["S_TOT"]], F16, kind="ExternalOutput")
        dbg_acc = nc.dram_tensor("dbg_acc", [ROWS, HID_F + 1], F32, kind="ExternalOutput")
        dbg_wm = nc.dram_tensor("dbg_wm", [128, CHUNK_SLOTS * 128], F16, kind="ExternalOutput")
        dbg_g = nc.dram_tensor("dbg_g", [128, CHUNK_SLOTS * 128], F16, kind="ExternalOutput")

    # inline constants (same for every core)
    def inl(name, arr):
        return nc.inline_tensor(np.ascontiguousarray(arr), name=name)

    c_W1a = inl("W1a", consts["W1"][:128].astype(np.float32))
    c_W1b = inl("W1b", consts["W1"][128:].astype(np.float32))
    c_R1 = inl("R1", consts["R1"])
    c_W2 = inl("W2", consts["W2"].astype(np.float32))
    c_R2 = inl("R2", consts["R2"])
    c_aw1t = inl("aw1t", consts["a1_w"][HID_F:].astype(np.float32).reshape(HID_F, 1))
    c_aw2t = inl("aw2t", consts["a2_w"][OUT_F:].astype(np.float32).reshape(OUT_F, 1))
    c_b1 = inl("b1c", consts["b1"].astype(np.float32).reshape(HID_F, 1))
    c_b1n = inl("b1n", (-consts["b1"]).astype(np.float32).reshape(HID_F, 1))
    c_b2 = inl("b2c", consts["b2"].astype(np.float32).reshape(OUT_F, 1))
    np_ed = np.float16 if f16 else np.float32
    c_iota = inl("iota", np.tile(np.arange(128, dtype=np_ed), (128, 1)))
    c_ones1 = inl("ones1", np.ones((1, 128), np.float32))
    c_onec = inl("onec", np.ones((128, 1), np_ed))
    c_I128 = inl("I128", np.eye(128, dtype=np.float32))
    c_I64 = inl("I64", np.eye(64, dtype=np.float32))
    c_b1a = inl("b1a", np.full((1, 1), consts["a1_b"][0], np.float32))
    c_b2a = inl("b2a", np.full((1, 1), consts["a2_b"][0], np.float32))
    c_nsh = inl("nsh", np.full((128, 1), -WSHIFT, np.float32))
    c1 = float(consts["c1"])
    c2 = float(consts["c2"])

    # internal DRAM
    h1p_sl = nc.dram_tensor("h1p_sl", [ROWS, TW1], F16)
    h1p_all = nc.dram_tensor("h1p_all", [TBL, TW1], F16, addr_space="Shared")
    h2p_sl = nc.dram_tensor("h2p_sl", [ROWS, TW2], F16)
    h2p_all = nc.dram_tensor("h2p_all", [TBL, TW2], F16, addr_space="Shared")

    def mid_bcast(ap, cs):  # [128, X] -> [128, cs(bcast), X]
        return bass.AP(ap.tensor, ap.offset, [list(ap.ap[0]), [0, cs], list(ap.ap[1])])

    def col0(ap3):  # [128, cs, tw] -> [128, cs] (element 0 of each row)
        return bass.AP(
            ap3.tensor, ap3.offset, [list(ap3.ap[0]), list(ap3.ap[1])]
        )

    with tile.TileContext(nc) as tc:
        import contextlib

        with contextlib.ExitStack() as ctx:
            cpool = ctx.enter_context(tc.tile_pool(name="consts", bufs=1))
            persist = ctx.enter_context(tc.tile_pool(name="persist", bufs=1))
            sb = ctx.enter_context(tc.tile_pool(name="sb", bufs=6))
            sb3 = ctx.enter_context(tc.tile_pool(name="sb3", bufs=6))
            ps = ctx.enter_context(tc.tile_pool(name="ps", bufs=3, space="PSUM"))
            psa = ctx.enter_context(tc.tile_pool(name="psa", bufs=2, space="PSUM"))

            def cload(handle, shape, dtype):
                t = cpool.tile(shape, dtype, tag=handle.name)
                nc.sync.dma_start(out=t[:], in_=handle[:, :])
                return t

            W1a = cload(c_W1a, [128, 128], F32)
            W1b = cload(c_W1b, [128, 128], F32)
            R1 = cload(c_R1, [128, 128], F32)
            W2 = cload(c_W2, [128, 64], F32)
            R2 = cload(c_R2, [64, 64], F32)
            aw1t = cload(c_aw1t, [128, 1], F32)
            aw2t = cload(c_aw2t, [64, 1], F32)
            b1c = cload(c_b1, [128, 1], F32)
            b1n = cload(c_b1n, [128, 1], F32)
            b2c = cload(c_b2, [64, 1], F32)
            iota = cload(c_iota, [128, 128], F16)
            ones1 = cload(c_ones1, [1, 128], F32)
            onec = cload(c_onec, [128, 1], F16)
            I128 = cload(c_I128, [128, 128], F32)
            I64 = cload(c_I64, [64, 64], F32)
            b1a = cload(c_b1a, [1, 1], F32)
            b2a = cload(c_b2a, [1, 1], F32)
            nsh = cload(c_nsh, [128, 1], F32)

            t1_sb = persist.tile([1, ROWS], F32, tag="t1")
            t2_sb = persist.tile([1, ROWS], F32, tag="t2")

            ll = nc.gpsimd.load_library(mlp_lib)
            nidx_regs = {}
            for (_o, _c, _n) in meta["call_cols"]:
                if _n not in nidx_regs:
                    _r = nc.gpsimd.alloc_register(f"nidx_{_n}")
                    nc.gpsimd.reg_mov(_r, _n)
                    nidx_regs[_n] = _r

            # ---------------- phase 1: h1' table + t1 ----------------
            for cix in range(NB):
                r0 = cix * 128
                xc = sb3.tile([128, IN_F], F32, tag="xc")
                nc.sync.dma_start(out=xc[:], in_=x_sl[r0:r0 + 128, :])
                xT0p = ps.tile([128, 128], F32, tag="pp")
                nc.tensor.transpose(xT0p[:], xc[:, 0:128], I128[:])
                xT1p = ps.tile([128, 128], F32, tag="pp")
                nc.tensor.transpose(xT1p[:], xc[:, 128:256], I128[:])
                xT0 = sb.tile([128, 128], F32, tag="xT")
                xT1 = sb.tile([128, 128], F32, tag="xT")
                nc.scalar.copy(xT0[:], xT0p[:])
                nc.scalar.copy(xT1[:], xT1p[:])
                hTp = ps.tile([128, 128], F32, tag="pp")
                nc.tensor.matmul(
                    hTp[:], lhsT=W1a[:], rhs=xT0[:], start=True, stop=False
                )
                nc.tensor.matmul(
                    hTp[:], lhsT=W1b[:], rhs=xT1[:], start=False, stop=True
                )
                ha = sb.tile([128, 128], F32, tag="ha")
                nc.scalar.activation(ha[:], hTp[:], AF.Relu, bias=b1c[:])
                hcn = sb.tile([128, 128], F32, tag="hcn")
                nc.scalar.activation(hcn[:], hTp[:], AF.Relu, bias=b1n[:], scale=-1.0)
                hdx = sb.tile([128, 128], F32, tag="hdx")
                nc.scalar.activation(hdx[:], hcn[:], AF.Exp, scale=-1.0)
                h1T = sb.tile([128, 128], F32, tag="h1T")
                nc.vector.scalar_tensor_tensor(
                    out=h1T[:], in0=hdx[:], scalar=-1.0, in1=ha[:],
                    op0=AL.add, op1=AL.add,
                )
                hpTp = ps.tile([128, 128], F32, tag="pp")
                nc.tensor.matmul(hpTp[:], lhsT=R1[:], rhs=h1T[:], start=True, stop=True)
                t1p = ps.tile([1, 128], F32, tag="pp")
                nc.tensor.matmul(t1p[:], lhsT=aw1t[:], rhs=h1T[:], start=True, stop=True)
                nc.scalar.activation(
                    t1_sb[0:1, r0:r0 + 128], t1p[:], AF.Identity, bias=b1a[:]
                )
                hpT = sb.tile([128, 128], F32, tag="hpT")
                nc.scalar.copy(hpT[:], hpTp[:])
                hrp = ps.tile([128, 128], F32, tag="pp")
                nc.tensor.transpose(hrp[:], hpT[:], I128[:])
                hrow = sb.tile([128, TW1], F16, tag="hrow")
                nc.scalar.copy(hrow[:], hrp[:])
                nc.sync.dma_start(out=h1p_sl[r0:r0 + 128, :], in_=hrow[:])

            nc.gpsimd.collective_compute(
                "AllGather",
                AL.bypass,
                replica_groups=[list(range(NCORES))],
                ins=[h1p_sl.ap().opt()],
                outs=[h1p_all.ap().opt()],
            )

            # ---------------- edge phase (both layers) ----------------
            def edge_layer(layer):
                if layer == 1:
                    table, tw, d, t_sb, cc, Rm = h1p_all, TW1, HID_F, t1_sb, c1, R1
                else:
                    table, tw, d, t_sb, cc, Rm = h2p_all, TW2, OUT_F, t2_sb, c2, R2
                colptr = [0]

                trow_all = persist.tile([128, NB * 128], F16, tag=f"trowall{layer}")
                for b in range(NB):
                    r0 = b * 128
                    trp = ps.tile([128, 128], F32, tag="pp")
                    nc.tensor.matmul(
                        trp[:], lhsT=ones1[:], rhs=t_sb[0:1, r0:r0 + 128],
                        start=True, stop=True,
                    )
                    nc.scalar.copy(trow_all[:, r0:r0 + 128], trp[:])

                for b in range(NB):
                    r0 = b * 128
                    trow = trow_all[:, r0:r0 + 128]

                    acc = psa.tile([128, d], F32, tag="acc")
                    accd = psa.tile([128, 1], F32, tag="accd")
                    chunks = meta["blocks"][b]
                    # batched per-block loads of idx columns and tgtrel
                    ncalls_b = sum(len(c[2]) for c in chunks)
                    c_lo = meta["call_cols"][colptr[0]][0]
                    c_hi_off, c_hi_n, _ = meta["call_cols"][colptr[0] + ncalls_b - 1]
                    c_hi = c_hi_off + c_hi_n
                    eib = sb.tile([128, 256], I16, tag="eib")
                    nc.sync.dma_start(out=eib[:, 0:c_hi - c_lo], in_=eidx[:, c_lo:c_hi])
                    sblk0 = int(meta["s_off"][b])
                    sblk1 = int(meta["s_off"][b + 1])
                    trb = sb.tile([128, 32], F16, tag="trb")
                    nc.sync.dma_start(
                        out=trb[:, 0:sblk1 - sblk0], in_=tgtrel[:, sblk0:sblk1]
                    )
                    n_chunks = len(chunks)
                    for ci, (c0, cs, calls) in enumerate(chunks):
                        s0 = int(meta["s_off"][b]) + c0
                        g = sb3.tile([128, CHUNK_SLOTS, tw], F16, tag="g")
                        for (sic, ncall, is_hi) in calls:
                            off, cols, n_idx = meta["call_cols"][colptr[0]]
                            colptr[0] += 1
                            tbl_ap = table[TH:TBL, :] if is_hi else table[:, :]
                            gi = nc.gpsimd.dma_gather(
                                g[:, sic:sic + ncall, :],
                                tbl_ap,
                                eib[:, off - c_lo:off - c_lo + cols],
                                num_idxs=n_idx,
                                num_idxs_reg=nidx_regs[n_idx],
                                elem_size=tw,
                                single_packet=False,
                                queue_num=colptr[0] % 4,
                            )
                            add_dep_helper(gi.ins, ll.ins)
                        delta = sb.tile([128, CHUNK_SLOTS, 128], F16, tag="delta")
                        nc.vector.tensor_tensor(
                            out=delta[:, 0:cs, :],
                            in0=trb[:, s0 - sblk0:s0 - sblk0 + cs].to_broadcast(
                                [128, cs, 128]
                            ),
                            in1=mid_bcast(iota[:], cs),
                            op=AL.subtract,
                        )
                        tsel = sb.tile([128, CHUNK_SLOTS, 128], F16, tag="tsel")
                        nc.vector.scalar_tensor_tensor(
                            out=tsel[:, 0:cs, :], in0=delta[:, 0:cs, :],
                            scalar=0.0, in1=mid_bcast(trow, cs),
                            op0=AL.is_equal, op1=AL.mult,
                        )
                        t_ed = sb.tile([128, CHUNK_SLOTS], F16, tag="t_ed")
                        with nc.allow_low_precision(
                            reason="one nonzero per segment; fp16 exact"
                        ):
                            nc.vector.tensor_reduce(
                                out=t_ed[:, 0:cs], in_=tsel[:, 0:cs, :],
                                axis=AX.X, op=AL.add,
                            )
                        z = sb.tile([128, CHUNK_SLOTS], F16, tag="z")
                        nc.vector.scalar_tensor_tensor(
                            out=z[:, 0:cs], in0=col0(g[:, 0:cs, :]), scalar=cc,
                            in1=t_ed[:, 0:cs], op0=AL.mult, op1=AL.add,
                        )
                        zl = sb.tile([128, CHUNK_SLOTS], F16, tag="zl")
                        nc.vector.scalar_tensor_tensor(
                            out=zl[:, 0:cs], in0=z[:, 0:cs], scalar=NEG,
                            in1=z[:, 0:cs], op0=AL.mult, op1=AL.max,
                        )
                        w = sb.tile([128, CHUNK_SLOTS], F16, tag="w")
                        nc.scalar.activation(
                            w[:, 0:cs], zl[:, 0:cs], AF.Exp, bias=nsh[:]
                        )
                        if debug and layer == 1:
                            nc.sync.dma_start(out=dbg_z[:, s0:s0 + cs], in_=z[:, 0:cs])
                            nc.sync.dma_start(out=dbg_ted[:, s0:s0 + cs], in_=t_ed[:, 0:cs])
                            nc.sync.dma_start(out=dbg_w[:, s0:s0 + cs], in_=w[:, 0:cs])
                        Wm = sb.tile([128, CHUNK_SLOTS, 128], F16, tag="Wm")
                        nc.vector.scalar_tensor_tensor(
                            out=Wm[:, 0:cs, :], in0=delta[:, 0:cs, :], scalar=0.0,
                            in1=w[:, 0:cs].to_broadcast([128, cs, 128]),
                            op0=AL.is_equal, op1=AL.mult,
                        )
                        if debug and layer == 1 and b == 0 and ci == 0:
                            nc.sync.dma_start(
                                out=dbg_wm[:, 0:cs * 128],
                                in_=Wm[:, 0:cs, :],
                            )
                            nc.sync.dma_start(
                                out=dbg_g[:, 0:cs * tw],
                                in_=g[:, 0:cs, :],
                            )
                        last_chunk = ci == n_chunks - 1
                        for j in range(cs):
                            st = (ci == 0) and (j == 0)
                            sp = last_chunk and (j == cs - 1)
                            nc.tensor.matmul(
                                acc[:], lhsT=Wm[:, j, :], rhs=g[:, j, 0:d],
                                start=st, stop=sp, skip_group_check=True,
                            )
                            nc.tensor.matmul(
                                accd[:], lhsT=Wm[:, j, :], rhs=onec[:],
                                start=st, stop=sp, skip_group_check=True,
                            )

                    if debug and layer == 1:
                        accsb = sb.tile([128, d + 1], F32, tag="accsb")
                        nc.scalar.copy(accsb[:, 0:d], acc[:])
                        nc.scalar.copy(accsb[:, d:d + 1], accd[:])
                        nc.sync.dma_start(out=dbg_acc[r0:r0 + 128, :], in_=accsb[:])
                    # -------- block epilogue --------
                    den = sb.tile([128, 1], F32, tag="den")
                    nc.vector.tensor_scalar_add(den[:], accd[:], EPS)
                    rec = sb.tile([128, 1], F32, tag="rec")
                    nc.vector.reciprocal(rec[:], den[:])
                    nrm = sb.tile([128, d], F32, tag="nrm")
                    nc.vector.tensor_tensor(
                        out=nrm[:], in0=acc[:],
                        in1=rec[:].to_broadcast([128, d]), op=AL.mult,
                    )
                    nTp = ps.tile([d, 128], F32, tag="pp")
                    nc.tensor.transpose(nTp[:], nrm[:], I128[:])
                    nT = sb.tile([d, 128], F32, tag="nT")
                    nc.scalar.copy(nT[:], nTp[:])
                    oTp = ps.tile([d, 128], F32, tag="pp")
                    nc.tensor.matmul(oTp[:], lhsT=Rm[:], rhs=nT[:], start=True, stop=True)

                    if layer == 1 and debug:
                        nc.sync.dma_start(out=dbg_nrm[r0:r0 + 128, :], in_=nrm[:])
                    if layer == 1:
                        oT = sb.tile([128, 128], F32, tag="o1T")
                        nc.scalar.copy(oT[:], oTp[:])
                        h2Tp = ps.tile([64, 128], F32, tag="pp")
                        nc.tensor.matmul(
                            h2Tp[:], lhsT=W2[:], rhs=oT[:], start=True, stop=True
                        )
                        h2T = sb.tile([64, 128], F32, tag="h2T")
                        nc.scalar.activation(h2T[:], h2Tp[:], AF.Identity, bias=b2c[:])
                        h2pTp = ps.tile([64, 128], F32, tag="pp")
                        nc.tensor.matmul(
                            h2pTp[:], lhsT=R2[:], rhs=h2T[:], start=True, stop=True
                        )
                        t2p = ps.tile([1, 128], F32, tag="pp")
                        nc.tensor.matmul(
                            t2p[:], lhsT=aw2t[:], rhs=h2T[:], start=True, stop=True
                        )
                        nc.scalar.activation(
                            t2_sb[0:1, r0:r0 + 128], t2p[:], AF.Identity, bias=b2a[:]
                        )
                        h2pT = sb.tile([64, 128], F32, tag="h2pT")
                        nc.scalar.copy(h2pT[:], h2pTp[:])
                        h2rp = ps.tile([128, 64], F32, tag="pp")
                        nc.tensor.transpose(h2rp[:], h2pT[:], I64[:])
                        h2r = sb.tile([128, TW2], F16, tag="h2r")
                        nc.scalar.copy(h2r[:, 0:64], h2rp[:])
                        if TW2 > 64:
                            nc.vector.memset(h2r[:, 64:TW2], 0.0)
                        nc.sync.dma_start(out=h2p_sl[r0:r0 + 128, :], in_=h2r[:])
                    else:
                        o2T = sb.tile([64, 128], F32, tag="o2T")
                        nc.scalar.copy(o2T[:], oTp[:])
                        o2p = ps.tile([128, 64], F32, tag="pp")
                        nc.tensor.transpose(o2p[:], o2T[:], I64[:])
                        o2 = sb.tile([128, 64], F32, tag="o2")
                        nc.scalar.copy(o2[:], o2p[:])
                        mx = sb.tile([128, 1], F32, tag="mx")
                        nc.vector.tensor_reduce(
                            out=mx[:], in_=o2[:], axis=AX.X, op=AL.max
                        )
                        mneg = sb.tile([128, 1], F32, tag="mneg")
                        nc.vector.tensor_scalar_mul(mneg[:], mx[:], -1.0)
                        ex = sb.tile([128, 64], F32, tag="ex")
                        nc.scalar.activation(ex[:], o2[:], AF.Exp, bias=mneg[:])
                        sm = sb.tile([128, 1], F32, tag="sm")
                        nc.vector.tensor_reduce(
                            out=sm[:], in_=ex[:], axis=AX.X, op=AL.add
                        )
                        ln = sb.tile([128, 1], F32, tag="ln")
                        nc.scalar.activation(ln[:], sm[:], AF.Ln)
                        mml = sb.tile([128, 1], F32, tag="mml")
                        nc.vector.tensor_tensor(
                            out=mml[:], in0=mx[:], in1=ln[:], op=AL.add
                        )
                        res = sb.tile([128, 64], F32, tag="res")
                        nc.vector.tensor_tensor(
                            out=res[:], in0=o2[:],
                            in1=mml[:].to_broadcast([128, 64]), op=AL.subtract,
                        )
                        nc.sync.dma_start(out=out_fin[r0:r0 + 128, :], in_=res[:])

            if debug:
                nc.sync.dma_start(out=dbg_h1[:, :], in_=h1p_all[:, :])
                nc.sync.dma_start(out=dbg_t1[:, :], in_=t1_sb[:, :])
            edge_layer(1)
            nc.gpsimd.collective_compute(
                "AllGather",
                AL.bypass,
                replica_groups=[list(range(NCORES))],
                ins=[h2p_sl.ap().opt()],
                outs=[h2p_all.ap().opt()],
            )
            if debug:
                nc.sync.dma_start(out=dbg_h2[:, :], in_=h2p_all[:, :])
            edge_layer(2)

    return nc


_CACHE = {}


def kernel(**inputs):
    from concourse.bass_utils import run_bass_kernel_spmd
    from concourse.library_overlay import lower_extended_insts

    x = np.ascontiguousarray(np.asarray(inputs["x"], np.float32))
    ei = np.asarray(inputs["edge_index"])
    meta, eidx, REL = prep_structures(ei)
    R1, c1 = _householder(np.asarray(inputs["a1_w"], np.float32)[:HID_F])
    R2, c2 = _householder(np.asarray(inputs["a2_w"], np.float32)[:OUT_F])
    consts = dict(
        W1=np.asarray(inputs["W1"], np.float32),
        b1=np.asarray(inputs["b1"], np.float32),
        W2=np.asarray(inputs["W2"], np.float32),
        b2=np.asarray(inputs["b2"], np.float32),
        a1_w=np.asarray(inputs["a1_w"], np.float32),
        a2_w=np.asarray(inputs["a2_w"], np.float32),
        a1_b=np.asarray(inputs["a1_b"], np.float32),
        a2_b=np.asarray(inputs["a2_b"], np.float32),
        R1=R1, R2=R2, c1=c1, c2=c2,
    )
    f16 = os.environ.get("GNN_F32", "0") != "1"
    nc = build_bass(meta, consts, f16=f16)
    _split_multi_waits(nc)
    lower_extended_insts(nc)

    np_ed = np.float16 if f16 else np.float32
    in_maps = []
    for k in range(NCORES):
        xs = np.zeros((ROWS, IN_F), np.float32)
        xs[:NPC] = x[k * NPC:(k + 1) * NPC]
        in_maps.append(
            {
                "x_sl": xs,
                "eidx": np.ascontiguousarray(eidx[k]),
                "tgtrel": np.ascontiguousarray(REL[k].astype(np_ed)),
            }
        )

    trace = os.environ.get("GNN_TRACE", "0") == "1"
    if trace:
        try:
            import types
            from trn_agent_boot.trn_boot import _ntff_profile_via_ctypes
            _h = _ntff_profile_via_ctypes("/opt/axon/libaxon_pjrt.so")
            m = types.ModuleType("antenv.axon_hooks")
            m.get_axon_ntff_profile_hook = lambda: _h
            sys.modules["antenv.axon_hooks"] = m
        except Exception as e:
            print("profile hook setup failed:", e)
            trace = False
    res = run_bass_kernel_spmd(
        nc, in_maps, core_ids=list(range(NCORES)), trace=trace
    )
    kernel.last_results = res
    out = np.concatenate(
        [res.results[k]["out_fin"][:NPC] for k in range(NCORES)], axis=0
    )
    return out.astype(np.float32)

